# revision 6
# baseline (speedup 1.0000x reference)
"""Trainium2 Bass kernel for nn_DistEstNet (DAGMM-style loss_fn).

Mathematical structure (validated against the fp32 reference):
  h     = tanh(X @ W1 + b1)                [N, H]
  gamma = sigmoid(h @ W2 + b2)             [N, K]
  The GMM energy term collapses to a constant in fp32: the Cholesky-diag
  product sqrt(det(2*pi*Sigma)) overflows fp32 (inf) for D=128, so
  mix == 0.0 exactly and max_val == 0.0 (quadratic forms are positive).
  Therefore  loss[n] = 0.2 * (-log(1e-12)) + 0.02 * sigma_diag  for all n,
  with sigma_diag = sum_{k,d} 1 / (B[k,d]/gs[k] - (A[k,d]/gs[k])^2)
  where gs = sum_n gamma, A = gamma^T X, B = gamma^T (X*X).

The device kernel computes gamma over all N (data-parallel over 8 cores),
accumulates [A | gs | B] in PSUM, all-reduces the [16,257] statistics
across cores, and broadcasts the resulting constant to the output shard.

v3 notes:
  - MLP1 and MLP2 run as fp8e4 DoubleRow matmuls (0.5 cycles/row).  MLP1
    splits D=128 into 64x2 on the partition axis; MLP2 splits H=512 into
    two 128x2 contractions.  tanh writes fp8 h directly.
  - Stats rhs rows are host-prepared as [X(128) | 1 | X^2(128)] (257 wide):
    one 257-col stats matmul per 128-sample block, no on-device squaring.
  - gammaT -> gamma transposition runs on DVE (32x32 block transpose, one
    instruction per group).  xt columns are host-permuted within each group
    (macro m takes samples 128*jj+32*m+nn) so the block-local transpose
    lands gamma exactly in the [128, 16] per-block stats lhsT layout.
"""

import time

import numpy as np
import ml_dtypes

import concourse.bacc as bacc
import concourse.tile as tile
from concourse import mybir
from concourse.bass_utils import run_bass_kernel_spmd

# Problem shape (hardcoded per spec)
N, D, H, K = 65536, 128, 512, 16
N_CORES = 8
NC = N // N_CORES          # 8192 samples per core
NBLK = NC // 128           # 64 blocks of 128 samples
NMAC = NC // 512           # 16 macro tiles of 512 samples
NPAIR = NMAC // 2          # 8 macro pairs
NGRP = NMAC // 4           # 4 groups of 4 macros (2048 samples)
SROW = 257                 # stats-rhs row from host: [X(128) | 1 | X^2(128)]
SFREE = 257                # stats columns: A=0:128, gs=128, B=129:257

# loss = LAMBDA_ENERGY * (-log(EPS_f32)) + LAMBDA_SIGMA * sigma_diag
C_ENERGY = float(np.float32(0.2) * np.float32(-np.log(np.float32(1e-12))))

BF16 = mybir.dt.float16
FP8 = mybir.dt.float8e4
F32 = mybir.dt.float32
AF = mybir.ActivationFunctionType
DR = mybir.MatmulPerfMode.DoubleRow


def _emit_main(tc, io, fast_bias):
    _emit_body(tc, io, fast_bias)
    _emit_tail(tc, io)


def _emit_body(tc, io, fast_bias):
    """MLP + stats accumulation + strip-sum into io['red_sb'].

    Software-pipelined: group g's gamma-side work (MLP2, sigmoid,
    DVE transpose, stats) is emitted interleaved between group (g+1)'s
    MLP1 macro-pairs so the ACT engine never starves on tanh."""
    nc = tc.nc
    xt_sb = io["xt_sb"]      # [64, 2*NC] fp8: D interleaved 64x2
    w1_sb = io["w1_sb"]      # [64, 1024] fp8: per c-chunk [64, 2, 128]
    w2_sb = io["w2_sb"]      # [128, 128] fp8: per chalf [128, 2, 32]
    b1c_sb = io["b1c_sb"]
    b2p_sb = io["b2p_sb"]
    xb_view = io["xb_view"]  # dram [g][128, 16*SROW]

    with (
        tc.tile_pool(name="xbg", bufs=3) as xbg_pool,
        tc.tile_pool(name="hTsb", bufs=7) as hTsb_pool,
        tc.tile_pool(name="gsb", bufs=2) as gsb_pool,
        tc.tile_pool(name="hTps", bufs=3, space="PSUM") as hTps_pool,
        tc.tile_pool(name="gps", bufs=1, space="PSUM") as gps_pool,
        tc.tile_pool(name="statsps", bufs=1, space="PSUM") as stats_pool,
    ):
        stats_ps = stats_pool.tile([128, SFREE], F32, tag="stats_ps")

        def emit_pair(g, pp):
            """MLP1 + tanh for macro pair pp (global); returns hT_sb tiles
            [chalf] each [128, 2048] fp8 = (c=2*chalf | c=2*chalf+1) x
            (macro 2pp | 2pp+1) halves of 512."""
            tiles = []
            for chalf in range(2):
                hT_sbt = hTsb_pool.tile([128, 2048], FP8, tag="hTsb")
                for cpar in range(2):
                    c = 2 * chalf + cpar
                    hT_ps = hTps_pool.tile([128, 1024], F32, tag="hTps")
                    for mm in range(2):
                        t = 2 * pp + mm
                        nc.tensor.matmul(
                            hT_ps[:, 512 * mm:512 * (mm + 1)],
                            w1_sb[:, 256 * c:256 * (c + 1)].rearrange(
                                "p (two f) -> p two f", two=2),
                            xt_sb[:, 1024 * t:1024 * (t + 1)].rearrange(
                                "p (two f) -> p two f", two=2),
                            start=True, stop=True,
                            perf_mode=DR,
                        )
                    if fast_bias:
                        nc.scalar.activation(
                            hT_sbt[:, 1024 * cpar:1024 * (cpar + 1)],
                            hT_ps[:], AF.Tanh)
                    else:
                        nc.scalar.activation(
                            hT_sbt[:, 1024 * cpar:1024 * (cpar + 1)],
                            hT_ps[:], AF.Tanh, bias=b1c_sb[:, c:c + 1])
                tiles.append(hT_sbt)
            return tiles

        def emit_mlp2(st):
            # plain fp8 matmuls (DoubleRow is illegal with col-offset dst
            # partitions: s3d3_mm_valid_dst_partition), col-tiled over the 4
            # macros so the streams overlap on hardware.
            g, hT_tiles, xbg = st
            gT_ps = gps_pool.tile([128, 512], F32, tag="gTps")
            for c in range(4):
                for m in range(4):
                    off = 1024 * (c % 2) + 512 * (m % 2)
                    rhs = hT_tiles[m // 2][c // 2][:, off:off + 512]
                    nc.tensor.matmul(
                        gT_ps[32 * m:32 * m + 32, :],
                        w2_sb[:, 32 * c:32 * (c + 1)],
                        rhs,
                        start=(c == 0), stop=(c == 3),
                        tile_position=(0, 32 * m),
                        skip_group_check=True,
                    )
            gT_sb = gsb_pool.tile([128, 512], BF16, tag="gTsb")
            nc.scalar.activation(gT_sb[:], gT_ps[:], AF.Sigmoid,
                                 bias=b2p_sb[:, 0:1])
            return gT_sb

        def emit_transpose(gT_sb):
            # DVE 32x32 block transpose: g_sb[32m+nn, 32jj+kk] =
            # gT_sb[32m+kk, 32jj+nn].  With the host-side xt column
            # permutation this makes g_sb[:, 32jj:32jj+16] the gamma
            # [128 samples, 16 clusters] lhsT for stats block jj.
            g_sb = gsb_pool.tile([128, 512], BF16, tag="gsb")
            nc.vector.transpose(g_sb[:], gT_sb[:])
            return g_sb

        def emit_stats(st, g_sb):
            g, hT_tiles, xbg = st
            for jj in range(16):
                s = jj % 4
                nc.tensor.matmul(
                    stats_ps[32 * s:32 * s + 16, 0:SFREE],
                    g_sb[:, 32 * jj:32 * jj + 16],
                    xbg[:, SROW * jj:SROW * (jj + 1)],
                    start=(g == 0 and jj < 4), stop=(g == NGRP - 1 and jj >= 12),
                    tile_position=(0, 32 * s),
                    skip_group_check=True,
                )

        pending = None
        pend_gT = None
        for g in range(NGRP):
            xbg = xbg_pool.tile([128, 16 * SROW], BF16, tag="xbg")
            nc.sync.dma_start(xbg[:], xb_view[g])

            hT_tiles = []
            hT_tiles.append(emit_pair(g, 2 * g))
            if pending is not None:
                pend_gT = emit_mlp2(pending)
                pend_g_sb = emit_transpose(pend_gT)
            hT_tiles.append(emit_pair(g, 2 * g + 1))
            if pending is not None:
                emit_stats(pending, pend_g_sb)
            pending = (g, hT_tiles, xbg)

        # flush last group
        gT_sb = emit_mlp2(pending)
        g_sb = emit_transpose(gT_sb)
        emit_stats(pending, g_sb)

        # strip-sum (DVE, valid partitions only)
        red_sb = io["red_sb"]
        nc.vector.tensor_copy(red_sb[:], stats_ps[0:16, :])
        for s in range(1, 4):
            nc.vector.tensor_add(red_sb[:], red_sb[:],
                                 stats_ps[32 * s:32 * s + 16, :])


def _emit_tail(tc, io):
    """All-reduce red_sb across cores, sigma_diag, broadcast to output."""
    nc = tc.nc
    one16_sb = io["one16_sb"]
    ones_out = io["ones_out"]
    out_view = io["out_view"]
    red_sb = io["red_sb"]
    with (
        tc.tile_pool(name="tail_sb", bufs=1) as tsb,
        tc.tile_pool(name="tail_ps", bufs=1, space="PSUM") as tps,
        tc.tile_pool(name="dram", bufs=1, space="DRAM") as dram,
    ):

        cc_in = dram.tile([16, SFREE], F32, tag="ccin")
        cc_out = dram.tile([16, SFREE], F32, tag="ccout")
        nc.gpsimd.dma_start(cc_in[:], red_sb[:])
        nc.gpsimd.collective_compute(
            "AllReduce", mybir.AluOpType.add,
            replica_groups=[list(range(N_CORES))],
            ins=[cc_in.opt()], outs=[cc_out.opt()],
        )
        ar_sb = tsb.tile([16, SFREE], F32, tag="ar")
        nc.gpsimd.dma_start(ar_sb[:], cc_out[:])

        rgs = tsb.tile([16, 1], F32, tag="rgs")
        nc.vector.reciprocal(rgs[:], ar_sb[:, 128:129])
        mu = tsb.tile([16, 128], F32, tag="mu")
        nc.vector.tensor_scalar_mul(mu[:], ar_sb[:, 0:128], rgs[:])
        var = tsb.tile([16, 128], F32, tag="var")
        nc.vector.tensor_scalar_mul(var[:], ar_sb[:, 129:257], rgs[:])
        mu2 = tsb.tile([16, 128], F32, tag="mu2")
        nc.vector.tensor_mul(mu2[:], mu[:], mu[:])
        nc.vector.tensor_sub(var[:], var[:], mu2[:])
        ivar = tsb.tile([16, 128], F32, tag="ivar")
        nc.vector.reciprocal(ivar[:], var[:])
        rowsum = tsb.tile([16, 1], F32, tag="rowsum")
        nc.vector.tensor_reduce(rowsum[:], ivar[:], axis=mybir.AxisListType.X,
                                op=mybir.AluOpType.add)

        sd_ps = tps.tile([128, 1], F32, tag="sd")
        nc.tensor.matmul(sd_ps[:], one16_sb[:], rowsum[:], start=True, stop=True)
        loss_sb = tsb.tile([128, 1], F32, tag="loss")
        nc.scalar.activation(loss_sb[:], sd_ps[:], AF.Copy,
                             bias=C_ENERGY, scale=0.02)
        out_sb = tsb.tile([128, 64], F32, tag="outsb")
        nc.vector.tensor_scalar_mul(out_sb[:], ones_out[:], loss_sb[:, 0:1])
        nc.sync.dma_start(out_view, out_sb[:])


def build(fast_bias=True, reps=1, single_core=False):
    """Build and compile the SPMD program. Returns the Bacc object."""
    nc = bacc.Bacc("TRN2", target_bir_lowering=False, debug=False,
                   num_devices=1 if single_core else N_CORES)

    xt_d = nc.dram_tensor("xt", [64, 2 * NC], FP8, kind="ExternalInput").ap()
    # host pre-permuted: [group][partition][block*SROW]
    xb_d = nc.dram_tensor("xb", [NGRP, 128, 16 * SROW], BF16,
                          kind="ExternalInput").ap()
    w1_d = nc.dram_tensor("w1", [64, 1024], FP8, kind="ExternalInput").ap()
    w2_d = nc.dram_tensor("w2", [128, 128], FP8, kind="ExternalInput").ap()
    b1c_d = nc.dram_tensor("b1c", [128, 4], F32, kind="ExternalInput").ap()
    b2p_d = nc.dram_tensor("b2p", [128, 1], F32, kind="ExternalInput").ap()
    one16_d = nc.dram_tensor("one16", [16, 128], F32, kind="ExternalInput").ap()
    out_d = nc.dram_tensor("out", [NC], F32, kind="ExternalOutput").ap()

    with tile.TileContext(nc) as tc:
        with tc.tile_pool(name="const", bufs=1) as const_pool:
            xt_sb = const_pool.tile([64, 2 * NC], FP8, tag="xt")
            w1_sb = const_pool.tile([64, 1024], FP8, tag="w1")
            w2_sb = const_pool.tile([128, 128], FP8, tag="w2")
            b1c_sb = const_pool.tile([128, 4], F32, tag="b1c")
            b2p_sb = const_pool.tile([128, 1], F32, tag="b2p")
            one16_sb = const_pool.tile([16, 128], F32, tag="one16")
            red_sb = const_pool.tile([16, SFREE], F32, tag="red_sb")
            ones_out = const_pool.tile([128, 64], F32, tag="onesout")

            nc.sync.dma_start(w1_sb[:], w1_d[:])
            nc.sync.dma_start(w2_sb[:], w2_d[:])
            nc.sync.dma_start(b1c_sb[:], b1c_d[:])
            nc.sync.dma_start(b2p_sb[:], b2p_d[:])
            nc.sync.dma_start(one16_sb[:], one16_d[:])
            nc.gpsimd.memset(ones_out[:], 1.0)
            for c in range(4):
                nc.sync.dma_start(xt_sb[:, 4096 * c:4096 * (c + 1)],
                                  xt_d[:, 4096 * c:4096 * (c + 1)])

            io = {
                "xt_sb": xt_sb, "w1_sb": w1_sb, "w2_sb": w2_sb,
                "b1c_sb": b1c_sb, "b2p_sb": b2p_sb,
                "one16_sb": one16_sb, "ones_out": ones_out,
                "red_sb": red_sb,
                "xb_view": xb_d,
                "out_view": out_d.rearrange("(p f) -> p f", p=128),
            }
            if isinstance(reps, tuple):  # dynamic loop variants for timing
                kind, R = reps
                if kind == "loop":      # loop everything incl collective+tail
                    with tc.For_i(0, R, 1):
                        _emit_main(tc, io, fast_bias)
                elif kind == "loopsr":  # same, staggered-reset back-edge
                    with tc.For_i(0, R, 1, staggered_reset=True):
                        _emit_main(tc, io, fast_bias)
                elif kind == "loopmain":  # loop main compute; tail once
                    with tc.For_i(0, R, 1):
                        _emit_body(tc, io, fast_bias)
                    _emit_tail(tc, io)
                elif kind == "loopmainsr":  # staggered-reset back-edge
                    with tc.For_i(0, R, 1, staggered_reset=True):
                        _emit_body(tc, io, fast_bias)
                    _emit_tail(tc, io)
                elif kind == "bodyonly":  # body only, dummy output (for TimelineSim)
                    for _ in range(R):
                        _emit_body(tc, io, fast_bias)
                    nc.sync.dma_start(io["out_view"], io["ones_out"][:])
                else:
                    raise ValueError(kind)
            else:
                for _ in range(reps):
                    _emit_main(tc, io, fast_bias)

    nc.compile()
    return nc


_PROGRAMS = {}


def _get_program(fast_bias, reps=1):
    key = (fast_bias, reps)
    if key not in _PROGRAMS:
        _PROGRAMS[key] = build(fast_bias, reps)
    return _PROGRAMS[key]


def make_in_maps(latent_samples, W1, b1, W2, b2):
    X = np.ascontiguousarray(np.asarray(latent_samples, dtype=np.float32))
    W1 = np.asarray(W1, dtype=np.float32)
    b1 = np.asarray(b1, dtype=np.float32)
    W2 = np.asarray(W2, dtype=np.float32)
    b2 = np.asarray(b2, dtype=np.float32)

    bf = np.float16
    f8 = ml_dtypes.float8_e4m3fn
    # w1 DoubleRow layout: per c-chunk [64, 2, 128] with D index d = p + 64*i
    w1dr = np.ascontiguousarray(
        W1.reshape(2, 64, 4, 128).transpose(1, 2, 0, 3)
    ).reshape(64, 1024).astype(f8)
    # w2: per c-chunk [128, 32] (K=16 padded to 32), H index = 128*c + p
    w2p = np.zeros((128, 4, 32), np.float32)
    w2p[:, :, :K] = W2.reshape(4, 128, K).transpose(1, 0, 2)
    w2dr = w2p.reshape(128, 128).astype(f8)
    b1c = np.ascontiguousarray(b1.reshape(4, 128).T)           # [128, 4] f32
    b2p = np.zeros((128, 1), np.float32)
    for m in range(4):
        b2p[32 * m:32 * m + 16, 0] = b2
    one16 = np.ones((16, 128), np.float32)

    in_maps = []
    for c in range(N_CORES):
        Xc = X[c * NC:(c + 1) * NC]                            # [8192, 128]
        # xt: within each group, macro m's column 32*jj+nn holds sample
        # 128*jj + 32*m + nn so the DVE block transpose of gammaT lands in
        # stats-block order.
        xp = Xc.reshape(NGRP, 16, 4, 32, D).transpose(0, 2, 1, 3, 4).reshape(NC, D)
        # DoubleRow rhs layout per macro: [64, 2, 512], D index d = p + 64*i
        xt = np.ascontiguousarray(
            xp.reshape(NMAC, 512, 2, 64).transpose(3, 0, 2, 1)
        ).reshape(64, 2 * NC).astype(f8)
        # xb rows: [X | 1 | X^2] (257), f32 squares cast to fp16
        xb = np.empty((NC, SROW), bf)
        xb[:, 0:128] = Xc.astype(bf)
        xb[:, 128] = np.asarray(1.0, bf)
        xb[:, 129:257] = (Xc * Xc).astype(bf)
        # permute to [group][partition][block*SROW] so each group's load is flat
        xb = np.ascontiguousarray(
            xb.reshape(NGRP, 16, 128, SROW).transpose(0, 2, 1, 3)
        ).reshape(NGRP, 128, 16 * SROW)
        in_maps.append({
            "xt": xt, "xb": xb, "w1": w1dr, "w2": w2dr,
            "b1c": b1c, "b2p": b2p, "one16": one16,
        })
    return in_maps, not np.any(b1)


def run(latent_samples, W1, b1, W2, b2, reps=1):
    in_maps, fast_bias = make_in_maps(latent_samples, W1, b1, W2, b2)
    nc = _get_program(fast_bias, reps)
    last_err = None
    for attempt in range(4):
        try:
            res = run_bass_kernel_spmd(nc, in_maps, list(range(N_CORES)))
            break
        except Exception as e:  # transient device wedge; retry
            last_err = e
            time.sleep(8)
    else:
        raise last_err
    out = np.concatenate([res.results[c]["out"] for c in range(N_CORES)])
    return out.astype(np.float32)


def kernel(latent_samples, W1, b1, W2, b2):
    return run(latent_samples, W1, b1, W2, b2, reps=1)


# revision 13
# speedup vs baseline: 1.1023x; 1.1023x over previous
"""Trainium2 Bass kernel for nn_DistEstNet (DAGMM-style loss_fn).

Mathematical structure (validated against the fp32 reference):
  h     = tanh(X @ W1 + b1)                [N, H]
  gamma = sigmoid(h @ W2 + b2)             [N, K]
  The GMM energy term collapses to a constant in fp32: the Cholesky-diag
  product sqrt(det(2*pi*Sigma)) overflows fp32 (inf) for D=128, so
  mix == 0.0 exactly and max_val == 0.0 (quadratic forms are positive).
  Therefore  loss[n] = 0.2 * (-log(1e-12)) + 0.02 * sigma_diag  for all n,
  with sigma_diag = sum_{k,d} 1 / (B[k,d]/gs[k] - (A[k,d]/gs[k])^2)
  where gs = sum_n gamma, A = gamma^T X, B = gamma^T (X*X).

The device kernel computes gamma over all N (data-parallel over 8 cores),
accumulates [A | gs | B] in PSUM, all-reduces the [16,257] statistics
across cores, and broadcasts the resulting constant to the output shard.

v3 notes:
  - MLP1 and MLP2 run as fp8e4 DoubleRow matmuls (0.5 cycles/row).  MLP1
    splits D=128 into 64x2 on the partition axis; MLP2 splits H=512 into
    two 128x2 contractions.  tanh writes fp8 h directly.
  - Stats rhs rows are host-prepared as [X(128) | 1 | X^2(128)] (257 wide):
    one 257-col stats matmul per 128-sample block, no on-device squaring.
  - gammaT -> gamma transposition runs on DVE (32x32 block transpose, one
    instruction per group).  xt columns are host-permuted within each group
    (macro m takes samples 128*jj+32*m+nn) so the block-local transpose
    lands gamma exactly in the [128, 16] per-block stats lhsT layout.
"""

import time

import numpy as np
import ml_dtypes

import concourse.bacc as bacc
import concourse.tile as tile
from concourse import mybir
from concourse.bass_utils import run_bass_kernel_spmd

# Problem shape (hardcoded per spec)
N, D, H, K = 65536, 128, 512, 16
N_CORES = 8
NC = N // N_CORES          # 8192 samples per core
NBLK = NC // 128           # 64 blocks of 128 samples
NMAC = NC // 512           # 16 macro tiles of 512 samples
NPAIR = NMAC // 2          # 8 macro pairs
NGRP = NMAC // 4           # 4 groups of 4 macros (2048 samples)
SROW = 257                 # stats-rhs row from host: [X(128) | 1 | X^2(128)]
SFREE = 257                # stats columns: A=0:128, gs=128, B=129:257

# loss = LAMBDA_ENERGY * (-log(EPS_f32)) + LAMBDA_SIGMA * sigma_diag
C_ENERGY = float(np.float32(0.2) * np.float32(-np.log(np.float32(1e-12))))

BF16 = mybir.dt.float16
FP8 = mybir.dt.float8e4
F32 = mybir.dt.float32
AF = mybir.ActivationFunctionType
DR = mybir.MatmulPerfMode.DoubleRow


def _emit_main(tc, io, fast_bias):
    _emit_body(tc, io, fast_bias)
    _emit_tail(tc, io)


def _emit_body(tc, io, fast_bias):
    """MLP + stats accumulation + strip-sum into io['red_sb'].

    Software-pipelined: group g's gamma-side work (MLP2, sigmoid,
    DVE transpose, stats) is emitted interleaved between group (g+1)'s
    MLP1 macro-pairs so the ACT engine never starves on tanh."""
    nc = tc.nc
    xt_sb = io["xt_sb"]      # [64, 2*NC] fp8: D interleaved 64x2
    w1_sb = io["w1_sb"]      # [64, 1024] fp8: per c-chunk [64, 2, 128]
    w2_sb = io["w2_sb"]      # [128, 128] fp8: per chalf [128, 2, 32]
    b1c_sb = io["b1c_sb"]
    b2p_sb = io["b2p_sb"]
    xb_view = io["xb_view"]  # dram [g][128, 16*SROW]

    with (
        tc.tile_pool(name="xbg", bufs=3) as xbg_pool,
        tc.tile_pool(name="hTsb", bufs=16) as hTsb_pool,
        tc.tile_pool(name="gsb", bufs=2) as gsb_pool,
        tc.tile_pool(name="hTps", bufs=2, space="PSUM") as hTps_pool,
        tc.tile_pool(name="gps", bufs=1, space="PSUM") as gps_pool,
        tc.tile_pool(name="statsps", bufs=1, space="PSUM") as stats_pool,
    ):
        stats_ps = stats_pool.tile([128, SFREE], F32, tag="stats_ps")

        # hT tiles: [128, 1536] = chunk c x macro-triple (3T, 3T+1, 3T+2);
        # the last triple T=5 only covers macro 15 (cols 0:512).
        def emit_triple(T):
            """MLP1 (fp8 DoubleRow) + tanh for macro triple T; returns the
            4 hT_sb tiles (one per H-chunk c)."""
            nmac = min(3, NMAC - 3 * T)
            width = 512 * nmac
            tiles = []
            for c in range(4):
                hT_ps = hTps_pool.tile([128, 1536], F32, tag="hTps")
                for mm in range(nmac):
                    t = 3 * T + mm
                    nc.tensor.matmul(
                        hT_ps[:, 512 * mm:512 * (mm + 1)],
                        w1_sb[:, 256 * c:256 * (c + 1)].rearrange(
                            "p (two f) -> p two f", two=2),
                        xt_sb[:, 1024 * t:1024 * (t + 1)].rearrange(
                            "p (two f) -> p two f", two=2),
                        start=True, stop=True,
                        perf_mode=DR,
                    )
                hT_sbt = hTsb_pool.tile([128, 1536], FP8, tag="hTsb")
                if fast_bias:
                    nc.scalar.activation(hT_sbt[:, 0:width],
                                         hT_ps[:, 0:width], AF.Tanh)
                else:
                    nc.scalar.activation(hT_sbt[:, 0:width],
                                         hT_ps[:, 0:width], AF.Tanh,
                                         bias=b1c_sb[:, c:c + 1])
                tiles.append(hT_sbt)
            return tiles

        def emit_mlp2(st):
            # plain fp8 matmuls (DoubleRow is illegal with col-offset dst
            # partitions: s3d3_mm_valid_dst_partition), col-tiled over the 4
            # macros so the streams overlap on hardware.
            g, trip_tiles, xbg = st
            gT_ps = gps_pool.tile([128, 512], F32, tag="gTps")
            for c in range(4):
                for m in range(4):
                    t = 4 * g + m
                    rhs = trip_tiles[t // 3][c][:, 512 * (t % 3):512 * (t % 3) + 512]
                    nc.tensor.matmul(
                        gT_ps[32 * m:32 * m + 32, :],
                        w2_sb[:, 32 * c:32 * (c + 1)],
                        rhs,
                        start=(c == 0), stop=(c == 3),
                        tile_position=(0, 32 * m),
                        skip_group_check=True,
                    )
            gT_sb = gsb_pool.tile([128, 512], BF16, tag="gTsb")
            nc.scalar.activation(gT_sb[:], gT_ps[:], AF.Sigmoid,
                                 bias=b2p_sb[:, 0:1])
            return gT_sb

        def emit_transpose(gT_sb, half):
            # DVE 32x32 block transpose: g_sb[32m+nn, 32jj+kk] =
            # gT_sb[32m+kk, 32jj+nn].  With the host-side xt column
            # permutation this makes g_sb[:, 32jj:32jj+16] the gamma
            # [128 samples, 16 clusters] lhsT for stats block jj.  Split in
            # halves so stats on blocks 0-7 overlap the second transpose.
            g_sb = gsb_pool.tile([128, 256], BF16, tag=f"gsb{half}")
            nc.vector.transpose(g_sb[:], gT_sb[:, 256 * half:256 * (half + 1)])
            return g_sb

        def emit_stats(st, g_sb, half):
            g, hT_tiles, xbg = st
            for jj8 in range(8):
                jj = 8 * half + jj8
                s = jj % 4
                nc.tensor.matmul(
                    stats_ps[32 * s:32 * s + 16, 0:SFREE],
                    g_sb[:, 32 * jj8:32 * jj8 + 16],
                    xbg[:, SROW * jj:SROW * (jj + 1)],
                    start=(g == 0 and jj < 4), stop=(g == NGRP - 1 and jj >= 12),
                    tile_position=(0, 32 * s),
                    skip_group_check=True,
                )

        # Schedule: MLP1 macro-triples T0..T5 (T5 partial), with group g's
        # gamma-side work interleaved as soon as its macros' tanh tiles
        # exist:  T0 T1 [g0:mlp2+tr] T2 [g0:stats; g1:mlp2+tr] T3
        # [g1:stats; g2:mlp2+tr] T4 [g2:stats] T5 [g3:all] stripsum.
        trip_tiles = {}
        xbgs = {}

        def fetch_xbg(g):
            xbg = xbg_pool.tile([128, 16 * SROW], BF16, tag="xbg")
            nc.sync.dma_start(xbg[:], xb_view[g])
            xbgs[g] = xbg

        def gamma_chain(g):
            """MLP2+sigmoid, then per-half transpose+stats so stats on
            blocks 0-7 overlap the second half's transpose."""
            st = (g, trip_tiles, xbgs[g])
            gT = emit_mlp2(st)
            for half in range(2):
                g_sb = emit_transpose(gT, half)
                emit_stats(st, g_sb, half)

        fetch_xbg(0)
        trip_tiles[0] = emit_triple(0)
        fetch_xbg(1)
        trip_tiles[1] = emit_triple(1)
        fetch_xbg(2)
        trip_tiles[2] = emit_triple(2)
        gamma_chain(0)
        fetch_xbg(3)
        trip_tiles[3] = emit_triple(3)
        gamma_chain(1)
        trip_tiles[4] = emit_triple(4)
        gamma_chain(2)
        trip_tiles[5] = emit_triple(5)
        gamma_chain(3)

        # strip-sum (DVE, valid partitions only; only one PSUM operand is
        # legal per DVE instruction)
        red_sb = io["red_sb"]
        nc.vector.tensor_copy(red_sb[:], stats_ps[0:16, :])
        for s in range(1, 4):
            nc.vector.tensor_add(red_sb[:], red_sb[:],
                                 stats_ps[32 * s:32 * s + 16, :])


def _emit_tail(tc, io):
    """All-reduce red_sb across cores, sigma_diag, broadcast to output."""
    nc = tc.nc
    one16_sb = io["one16_sb"]
    ones_out = io["ones_out"]
    out_view = io["out_view"]
    red_sb = io["red_sb"]
    with (
        tc.tile_pool(name="tail_sb", bufs=1) as tsb,
        tc.tile_pool(name="tail_ps", bufs=1, space="PSUM") as tps,
        tc.tile_pool(name="dram", bufs=1, space="DRAM") as dram,
    ):

        cc_in = dram.tile([16, SFREE], F32, tag="ccin")
        cc_out = dram.tile([16, SFREE], F32, tag="ccout")
        nc.gpsimd.dma_start(cc_in[:], red_sb[:])
        nc.gpsimd.collective_compute(
            "AllReduce", mybir.AluOpType.add,
            replica_groups=[list(range(N_CORES))],
            ins=[cc_in.opt()], outs=[cc_out.opt()],
        )
        ar_sb = tsb.tile([16, SFREE], F32, tag="ar")
        nc.gpsimd.dma_start(ar_sb[:], cc_out[:])

        rgs = tsb.tile([16, 1], F32, tag="rgs")
        nc.vector.reciprocal(rgs[:], ar_sb[:, 128:129])
        mu = tsb.tile([16, 128], F32, tag="mu")
        nc.vector.tensor_scalar_mul(mu[:], ar_sb[:, 0:128], rgs[:])
        var = tsb.tile([16, 128], F32, tag="var")
        nc.vector.tensor_scalar_mul(var[:], ar_sb[:, 129:257], rgs[:])
        mu2 = tsb.tile([16, 128], F32, tag="mu2")
        nc.vector.tensor_mul(mu2[:], mu[:], mu[:])
        nc.vector.tensor_sub(var[:], var[:], mu2[:])
        ivar = tsb.tile([16, 128], F32, tag="ivar")
        nc.vector.reciprocal(ivar[:], var[:])
        rowsum = tsb.tile([16, 1], F32, tag="rowsum")
        nc.vector.tensor_reduce(rowsum[:], ivar[:], axis=mybir.AxisListType.X,
                                op=mybir.AluOpType.add)

        sd_ps = tps.tile([128, 1], F32, tag="sd")
        nc.tensor.matmul(sd_ps[:], one16_sb[:], rowsum[:], start=True, stop=True)
        loss_sb = tsb.tile([128, 1], F32, tag="loss")
        nc.scalar.activation(loss_sb[:], sd_ps[:], AF.Copy,
                             bias=C_ENERGY, scale=0.02)
        out_sb = tsb.tile([128, 64], F32, tag="outsb")
        nc.vector.tensor_scalar_mul(out_sb[:], ones_out[:], loss_sb[:, 0:1])
        nc.sync.dma_start(out_view, out_sb[:])


def build(fast_bias=True, reps=1, single_core=False):
    """Build and compile the SPMD program. Returns the Bacc object."""
    nc = bacc.Bacc("TRN2", target_bir_lowering=False, debug=False,
                   num_devices=1 if single_core else N_CORES)

    xt_d = nc.dram_tensor("xt", [64, 2 * NC], FP8, kind="ExternalInput").ap()
    # host pre-permuted: [group][partition][block*SROW]
    xb_d = nc.dram_tensor("xb", [NGRP, 128, 16 * SROW], BF16,
                          kind="ExternalInput").ap()
    w1_d = nc.dram_tensor("w1", [64, 1024], FP8, kind="ExternalInput").ap()
    w2_d = nc.dram_tensor("w2", [128, 128], FP8, kind="ExternalInput").ap()
    b1c_d = nc.dram_tensor("b1c", [128, 4], F32, kind="ExternalInput").ap()
    b2p_d = nc.dram_tensor("b2p", [128, 1], F32, kind="ExternalInput").ap()
    one16_d = nc.dram_tensor("one16", [16, 128], F32, kind="ExternalInput").ap()
    out_d = nc.dram_tensor("out", [NC], F32, kind="ExternalOutput").ap()

    with tile.TileContext(nc) as tc:
        with tc.tile_pool(name="const", bufs=1) as const_pool:
            xt_sb = const_pool.tile([64, 2 * NC], FP8, tag="xt")
            w1_sb = const_pool.tile([64, 1024], FP8, tag="w1")
            w2_sb = const_pool.tile([128, 128], FP8, tag="w2")
            b1c_sb = const_pool.tile([128, 4], F32, tag="b1c")
            b2p_sb = const_pool.tile([128, 1], F32, tag="b2p")
            one16_sb = const_pool.tile([16, 128], F32, tag="one16")
            red_sb = const_pool.tile([16, SFREE], F32, tag="red_sb")
            ones_out = const_pool.tile([128, 64], F32, tag="onesout")

            nc.sync.dma_start(w1_sb[:], w1_d[:])
            nc.sync.dma_start(w2_sb[:], w2_d[:])
            nc.sync.dma_start(b1c_sb[:], b1c_d[:])
            nc.sync.dma_start(b2p_sb[:], b2p_d[:])
            nc.sync.dma_start(one16_sb[:], one16_d[:])
            nc.gpsimd.memset(ones_out[:], 1.0)
            for c in range(4):
                nc.sync.dma_start(xt_sb[:, 4096 * c:4096 * (c + 1)],
                                  xt_d[:, 4096 * c:4096 * (c + 1)])

            io = {
                "xt_sb": xt_sb, "w1_sb": w1_sb, "w2_sb": w2_sb,
                "b1c_sb": b1c_sb, "b2p_sb": b2p_sb,
                "one16_sb": one16_sb, "ones_out": ones_out,
                "red_sb": red_sb,
                "xb_view": xb_d,
                "out_view": out_d.rearrange("(p f) -> p f", p=128),
            }
            if isinstance(reps, tuple):  # dynamic loop variants for timing
                kind, R = reps
                if kind == "loop":      # loop everything incl collective+tail
                    with tc.For_i(0, R, 1):
                        _emit_main(tc, io, fast_bias)
                elif kind == "loopsr":  # same, staggered-reset back-edge
                    with tc.For_i(0, R, 1, staggered_reset=True):
                        _emit_main(tc, io, fast_bias)
                elif kind == "loopmain":  # loop main compute; tail once
                    with tc.For_i(0, R, 1):
                        _emit_body(tc, io, fast_bias)
                    _emit_tail(tc, io)
                elif kind == "loopmainsr":  # staggered-reset back-edge
                    with tc.For_i(0, R, 1, staggered_reset=True):
                        _emit_body(tc, io, fast_bias)
                    _emit_tail(tc, io)
                elif kind == "bodyonly":  # body only, dummy output (for TimelineSim)
                    for _ in range(R):
                        _emit_body(tc, io, fast_bias)
                    nc.sync.dma_start(io["out_view"], io["ones_out"][:])
                else:
                    raise ValueError(kind)
            else:
                for _ in range(reps):
                    _emit_main(tc, io, fast_bias)

    nc.compile()
    return nc


_PROGRAMS = {}


def _get_program(fast_bias, reps=1):
    key = (fast_bias, reps)
    if key not in _PROGRAMS:
        _PROGRAMS[key] = build(fast_bias, reps)
    return _PROGRAMS[key]


def make_in_maps(latent_samples, W1, b1, W2, b2):
    X = np.ascontiguousarray(np.asarray(latent_samples, dtype=np.float32))
    W1 = np.asarray(W1, dtype=np.float32)
    b1 = np.asarray(b1, dtype=np.float32)
    W2 = np.asarray(W2, dtype=np.float32)
    b2 = np.asarray(b2, dtype=np.float32)

    bf = np.float16
    f8 = ml_dtypes.float8_e4m3fn
    # w1 DoubleRow layout: per c-chunk [64, 2, 128] with D index d = p + 64*i
    w1dr = np.ascontiguousarray(
        W1.reshape(2, 64, 4, 128).transpose(1, 2, 0, 3)
    ).reshape(64, 1024).astype(f8)
    # w2: per c-chunk [128, 32] (K=16 padded to 32), H index = 128*c + p
    w2p = np.zeros((128, 4, 32), np.float32)
    w2p[:, :, :K] = W2.reshape(4, 128, K).transpose(1, 0, 2)
    w2dr = w2p.reshape(128, 128).astype(f8)
    b1c = np.ascontiguousarray(b1.reshape(4, 128).T)           # [128, 4] f32
    b2p = np.zeros((128, 1), np.float32)
    for m in range(4):
        b2p[32 * m:32 * m + 16, 0] = b2
    one16 = np.ones((16, 128), np.float32)

    in_maps = []
    for c in range(N_CORES):
        Xc = X[c * NC:(c + 1) * NC]                            # [8192, 128]
        # xt: within each group, macro m's column 32*jj+nn holds sample
        # 128*jj + 32*m + nn so the DVE block transpose of gammaT lands in
        # stats-block order.
        xp = Xc.reshape(NGRP, 16, 4, 32, D).transpose(0, 2, 1, 3, 4).reshape(NC, D)
        # DoubleRow rhs layout per macro: [64, 2, 512], D index d = p + 64*i
        xt = np.ascontiguousarray(
            xp.reshape(NMAC, 512, 2, 64).transpose(3, 0, 2, 1)
        ).reshape(64, 2 * NC).astype(f8)
        # xb rows: [X | 1 | X^2] (257), f32 squares cast to fp16
        xb = np.empty((NC, SROW), bf)
        xb[:, 0:128] = Xc.astype(bf)
        xb[:, 128] = np.asarray(1.0, bf)
        xb[:, 129:257] = (Xc * Xc).astype(bf)
        # permute to [group][partition][block*SROW] so each group's load is flat
        xb = np.ascontiguousarray(
            xb.reshape(NGRP, 16, 128, SROW).transpose(0, 2, 1, 3)
        ).reshape(NGRP, 128, 16 * SROW)
        in_maps.append({
            "xt": xt, "xb": xb, "w1": w1dr, "w2": w2dr,
            "b1c": b1c, "b2p": b2p, "one16": one16,
        })
    return in_maps, not np.any(b1)


def run(latent_samples, W1, b1, W2, b2, reps=1):
    in_maps, fast_bias = make_in_maps(latent_samples, W1, b1, W2, b2)
    nc = _get_program(fast_bias, reps)
    last_err = None
    for attempt in range(4):
        try:
            res = run_bass_kernel_spmd(nc, in_maps, list(range(N_CORES)))
            break
        except Exception as e:  # transient device wedge; retry
            last_err = e
            time.sleep(8)
    else:
        raise last_err
    out = np.concatenate([res.results[c]["out"] for c in range(N_CORES)])
    return out.astype(np.float32)


def kernel(latent_samples, W1, b1, W2, b2):
    return run(latent_samples, W1, b1, W2, b2, reps=1)
